# revision 25
# baseline (speedup 1.0000x reference)
"""Trainium2 Bass kernel for nn_AudioSelfAttention (B=2, T=2048, C=1024, H=16).

Sharding: batch x head-group tensor parallel. Core i handles batch i//4 and
heads 4*(i%4)..4*(i%4)+3 (2 head pairs) over the full 2048-token batch.
Each core computes q/k/v only for its own 4 heads (no redundant K/V work),
runs attention for its heads over all queries, and produces a partial
output projection out_partial = y_heads @ W_proj[head rows, :] [2048,1024].
The host sums the 4 partials per batch in fp32 (the unshard step).

Attention unit = (head pair, 512-query tile); 8 units cover the 2 pairs x
2048 queries. Per key chunk: S^T via a row-tiled pair of 64-contraction
matmuls (concurrent on disjoint PE row groups; the full 2x2 row+col split
would halve S time but PE quadrant (64,64) is broken on trn2), exp on
ScalarE over [128,1024] PSUM spans, y^T = P^T V with the exp tile
stationary and V+ones moving (65-col matmuls, sumexp in column 64).

ScalarE's exp stream is the bottleneck engine (~1.05us per chunk when the
PE is lightly loaded, ~1.26us when saturated — keep mid-stream PE drips
moderate). K(p0)+Q(p0,0)+V(0..7) run in a dense prologue at full PE rate;
V(8..15), K(p1), the other Q tiles and the output projection are dripped
into the attention units' PE slack. Out-DMAs ride the GpSimd queue: a DMA
wait parked on the scalar queue blocks the exp stream behind it.

Compute dtype: bf16 matmul operands, fp32 PSUM accumulation and softmax
statistics; v/proj biases folded exactly on the host (softmax rows sum to
1); q/k biases applied on-device in the PSUM->SBUF copies.
"""
import numpy as np

_CACHE = {}

B, T, C, H, D = 2, 2048, 1024, 16, 64
CC = C // 128            # 8 contraction chunks
NKT = T // 128           # 16 key chunks
TQ = 512                 # query tile


def _build_nc():
    import concourse.bacc as bacc
    import concourse.tile as tile
    import concourse.mybir as mybir

    f32 = mybir.dt.float32
    bf16 = mybir.dt.bfloat16
    Exp = mybir.ActivationFunctionType.Exp
    Copy = mybir.ActivationFunctionType.Copy

    nc = bacc.Bacc(None, num_devices=8)
    xt = nc.declare_dram_parameter("xt", [128, 4, CC, TQ], bf16, isOutput=False)
    # jc 0,1 = q chunks (pair0, pair1), 2,3 = k, 4,5 = v
    wqkv = nc.declare_dram_parameter("wqkv", [128, 6, CC, 128], bf16, isOutput=False)
    bqk = nc.declare_dram_parameter("bqk", [128, 4], f32, isOutput=False)
    wp = nc.declare_dram_parameter("wp", [128, 2, C], bf16, isOutput=False)
    eye = nc.declare_dram_parameter("eye", [128, 128], bf16, isOutput=False)
    out = nc.declare_dram_parameter("out", [T, C], bf16, isOutput=True)

    with tile.TileContext(nc) as tc:
        with (
            tc.tile_pool(name="big", bufs=1) as big,
            tc.tile_pool(name="pexp", bufs=8) as pexp,
            tc.tile_pool(name="small", bufs=2) as small,
            tc.tile_pool(name="opool", bufs=6) as opool,
            tc.tile_pool(name="mmps", bufs=2, space="PSUM") as mmps,
            tc.tile_pool(name="spool", bufs=2, space="PSUM") as spool,
            tc.tile_pool(name="ypool", bufs=2, space="PSUM") as ypool,
        ):
            # ---- persistent SBUF tensors; DMAs in consumption order.
            # sync: eye, xt0, xt1, wv, xt2, xt3. scalar: wk0, wq0, bqk,
            # wk1, wq1, wp. (wv early — it gates the prologue V tiles.)
            eye_sb = big.tile([128, 128], bf16)
            nc.sync.dma_start(eye_sb[:], eye[:])
            xt_sb = big.tile([128, 4, CC, TQ], bf16)
            nc.sync.dma_start(xt_sb[:, 0], xt[:, 0])
            wqkv_sb = big.tile([128, 6, CC, 128], bf16)
            nc.scalar.dma_start(wqkv_sb[:, 2:3], wqkv[:, 2:3])
            nc.scalar.dma_start(wqkv_sb[:, 0:1], wqkv[:, 0:1])
            bqk_sb = big.tile([128, 4], f32)
            nc.scalar.dma_start(bqk_sb[:], bqk[:])
            nc.gpsimd.dma_start(wqkv_sb[:, 4:6], wqkv[:, 4:6])
            nc.sync.dma_start(xt_sb[:, 1], xt[:, 1])
            nc.scalar.dma_start(wqkv_sb[:, 3:4], wqkv[:, 3:4])
            nc.scalar.dma_start(wqkv_sb[:, 1:2], wqkv[:, 1:2])
            nc.gpsimd.dma_start(xt_sb[:, 2], xt[:, 2])
            nc.gpsimd.dma_start(xt_sb[:, 3], xt[:, 3])
            wp_sb = big.tile([128, 2, C], bf16)
            nc.scalar.dma_start(wp_sb[:], wp[:])

            q_sb = big.tile([128, 2, T], bf16)
            k_sb = big.tile([128, 2, T], bf16)
            v_sb = big.tile([128, NKT, 4, 72], bf16)
            nc.vector.memset(v_sb[:, :, :, 64:65], 1.0)
            yt_sb = big.tile([128, 2, T], bf16)

            # ---- PE warm-up on uninitialized SBUF during the DMA lead-in
            warm_ps = mmps.tile([128, TQ], f32, tag="mm", name="warm")
            for w in range(20):
                nc.tensor.matmul(warm_ps[:], yt_sb[0:128, 0, 0:128],
                                 yt_sb[:, w % 2, 0:TQ],
                                 start=(w == 0), stop=(w == 19),
                                 skip_group_check=True)

            # ---- K(pair0) ----
            def emit_k_tt(p, tt):
                kps = mmps.tile([128, TQ], f32, tag="mm", name=f"k{p}_{tt}")
                for cc in range(CC):
                    nc.tensor.matmul(kps[:], wqkv_sb[:, 2 + p, cc, :],
                                     xt_sb[:, tt, cc, :],
                                     start=(cc == 0), stop=(cc == CC - 1))
                nc.vector.tensor_scalar_add(
                    k_sb[:, p, tt * TQ:(tt + 1) * TQ], kps[:],
                    bqk_sb[:, 2 + p:3 + p])

            for tt in range(3):
                emit_k_tt(0, tt)

            # ---- Q tiles: 2 accumulation matmuls per call ----
            def emit_q_cc2(p, qt, cc0, st):
                if cc0 == 0:
                    st["ps"] = mmps.tile([128, TQ], f32, tag="mm",
                                         name=f"q{p}_{qt}")
                for cc in (cc0, cc0 + 1):
                    nc.tensor.matmul(st["ps"][:], wqkv_sb[:, p, cc, :],
                                     xt_sb[:, qt, cc, :],
                                     start=(cc == 0), stop=(cc == CC - 1))
                if cc0 == CC - 2:
                    nc.vector.tensor_scalar_add(
                        q_sb[:, p, qt * TQ:(qt + 1) * TQ], st["ps"][:],
                        bqk_sb[:, p:p + 1])

            st0 = {}
            for cc0 in range(0, CC, 2):
                emit_q_cc2(0, 0, cc0, st0)

            # ---- V tiles (joint 256-wide): 0..7 here, 8..15 dripped ----
            def emit_v(tc_i):
                vps = mmps.tile([128, 2, 128], f32, tag="mm", name=f"v{tc_i}")
                for cc in range(CC):
                    nc.tensor.matmul(
                        vps[:],
                        xt_sb[:, tc_i // 4, cc,
                              (tc_i % 4) * 128:(tc_i % 4) * 128 + 128],
                        wqkv_sb[:, 4:6, cc, :],
                        start=(cc == 0), stop=(cc == CC - 1))
                nc.vector.tensor_copy(
                    v_sb[:, tc_i, :, 0:64],
                    vps.rearrange("p j (h f) -> p (j h) f", h=2))

            for tc_i in range(6):
                emit_v(tc_i)

            # ---- attention: 8 units, pair-major (p0 qt0..3, p1 qt0..3) ----
            for u in range(8):
                p, qt = u // 4, u % 4
                qoff = qt * TQ
                last = u == 7

                yA = ypool.tile([128, 4, 66], f32, tag="y", name=f"yA_{u}")
                yB = ypool.tile([128, 4, 66], f32, tag="y", name=f"yB_{u}")
                pe_tiles = {}

                def emit_y(c, yA=yA, yB=yB, p=p):
                    # start=True clears the whole PSUM bank's has_written;
                    # only the first matmul into the bank sets it
                    pe_t = pe_tiles.pop(c)
                    for qc in range(4):
                        nc.tensor.matmul(yA[:, qc, 0:65],
                                         pe_t[:, qc * 128:(qc + 1) * 128],
                                         v_sb[:, c, 2 * p, 0:65],
                                         start=(c == 0 and qc == 0),
                                         stop=(c == NKT - 1 and qc == 3),
                                         skip_group_check=True)
                        nc.tensor.matmul(yB[:, qc, 0:65],
                                         pe_t[:, TQ + qc * 128:TQ + (qc + 1) * 128],
                                         v_sb[:, c, 2 * p + 1, 0:65],
                                         start=(c == 0 and qc == 0),
                                         stop=(c == NKT - 1 and qc == 3),
                                         skip_group_check=True)

                # drip work for this unit, consumed 2 slots per odd kc
                drips = []
                if u == 0:
                    # K(p0,tt3) first (needed by S(12)), then V tiles 6..15;
                    # V(tc) must land before y(tc) (3-chunk lag)
                    stk0 = {}
                    for cc0 in range(0, CC, 2):
                        drips.append(
                            lambda cc0=cc0, stk0=stk0: _emit_k_cc2(
                                nc, mmps, wqkv_sb, xt_sb, k_sb, bqk_sb,
                                0, 3, cc0, stk0))
                    for j in range(10):
                        drips.append(lambda j=j: emit_v(6 + j))
                elif u in (1, 2):
                    # K(pair1): 2 tt per unit, 8 cc matmuls over 4 slots
                    for tt in (2 * (u - 1), 2 * (u - 1) + 1):
                        stk = {}
                        for cc0 in range(0, CC, 2):
                            drips.append(
                                lambda tt=tt, cc0=cc0, stk=stk: _emit_k_cc2(
                                    nc, mmps, wqkv_sb, xt_sb, k_sb, bqk_sb,
                                    1, tt, cc0, stk))
                if u < 7:
                    pn, qn = (u + 1) // 4, (u + 1) % 4
                    stq = {}
                    for cc0 in range(0, CC, 2):
                        drips.append(
                            lambda pn=pn, qn=qn, cc0=cc0, stq=stq:
                            emit_q_cc2(pn, qn, cc0, stq))
                if u >= 5:
                    # proj for qt u-5 (yt cc0+cc1 complete after unit 4+qt)
                    qp = u - 5
                    for ti in range(4):
                        for oh in range(2):
                            drips.append(
                                lambda qp=qp, ti=ti, oh=oh: _emit_proj(
                                    nc, mmps, opool, yt_sb, wp_sb, out,
                                    qp, ti, oh))

                di = iter(drips)

                def drip():
                    fn = next(di, None)
                    if fn is not None:
                        fn()

                for c in range(NKT):
                    koff = c * 128
                    sp = spool.tile([128, 2 * TQ], f32, tag="s",
                                    name=f"s_{u}_{c}")
                    nc.tensor.matmul(sp[:, 0:TQ],
                                     k_sb[0:64, p, koff:koff + 128],
                                     q_sb[0:64, p, qoff:qoff + TQ],
                                     start=True, stop=True)
                    nc.tensor.matmul(sp[:, TQ:2 * TQ],
                                     k_sb[64:128, p, koff:koff + 128],
                                     q_sb[64:128, p, qoff:qoff + TQ],
                                     start=True, stop=True)
                    pe_t = pexp.tile([128, 2 * TQ], bf16, tag="pe",
                                     name=f"pe_{u}_{c}")
                    nc.scalar.activation(pe_t[:], sp[:], Exp, scale=0.125)
                    pe_tiles[c] = pe_t
                    if c % 2 == 1:
                        if c >= 3:
                            emit_y(c - 3)
                            emit_y(c - 2)
                        drip()
                        drip()
                emit_y(NKT - 2)
                emit_y(NKT - 1)
                for _ in range(4):
                    drip()

                # normalization + transpose back to [dim, query]
                rr = small.tile([128, 2, 4], f32, tag="rr", name=f"rr_{u}")
                nc.vector.reciprocal_approx_fast(rr[:, 0, :], yA[:, :, 64])
                nc.vector.reciprocal_approx_fast(rr[:, 1, :], yB[:, :, 64])
                yn = small.tile([128, 4, 128], bf16, tag="yn", name=f"yn_{u}")
                for qc in range(4):
                    if last:
                        nc.scalar.activation(yn[:, qc, 0:64], yA[:, qc, 0:64],
                                             Copy, scale=rr[:, 0, qc:qc + 1])
                    else:
                        nc.vector.tensor_scalar_mul(yn[:, qc, 0:64],
                                                    yA[:, qc, 0:64],
                                                    rr[:, 0, qc:qc + 1])
                    nc.vector.tensor_scalar_mul(yn[:, qc, 64:128],
                                                yB[:, qc, 0:64],
                                                rr[:, 1, qc:qc + 1])
                    if not last:
                        nc.sync.dma_start_transpose(
                            yt_sb[:, p, qoff + qc * 128:qoff + (qc + 1) * 128],
                            yn[:, qc, :])
                if last:
                    # tail: PE transposes + proj qt3 interleaved per qc;
                    # cc0-half proj matmuls pre-run while norm muls drain
                    early = {}
                    for qc in (0, 1):
                        pr = []
                        for oh in range(2):
                            o_ps = (mmps if oh == 0 else spool).tile(
                                [128, TQ], f32, tag="mm" if oh == 0 else "s",
                                name=f"op_{qc}_{oh}")
                            nc.tensor.matmul(
                                o_ps[:],
                                yt_sb[:, 0, 1536 + qc * 128:1536 + (qc + 1) * 128],
                                wp_sb[:, 0, oh * TQ:(oh + 1) * TQ],
                                start=True, stop=False)
                            pr.append(o_ps)
                        early[qc] = pr
                    qdma = [nc.sync, nc.gpsimd, nc.scalar, nc.gpsimd]
                    for qc in range(4):
                        tp = ypool.tile([128, 128], bf16, tag="y",
                                        name=f"tp_{qc}")
                        nc.tensor.transpose(tp[:], yn[:, qc, :], eye_sb[:])
                        nc.scalar.activation(
                            yt_sb[:, 1, 1536 + qc * 128:1536 + (qc + 1) * 128],
                            tp[:], Copy)
                        if qc in early:
                            pr = early.pop(qc)
                        else:
                            pr = []
                            for oh in range(2):
                                o_ps = (mmps if oh == 0 else spool).tile(
                                    [128, TQ], f32,
                                    tag="mm" if oh == 0 else "s",
                                    name=f"op_{qc}_{oh}")
                                nc.tensor.matmul(
                                    o_ps[:],
                                    yt_sb[:, 0, 1536 + qc * 128:1536 + (qc + 1) * 128],
                                    wp_sb[:, 0, oh * TQ:(oh + 1) * TQ],
                                    start=True, stop=False)
                                pr.append(o_ps)
                        o_sb = small.tile([128, 2, TQ], bf16, tag="osb",
                                          name=f"osb_{qc}")
                        for oh in range(2):
                            nc.tensor.matmul(
                                pr[oh][:],
                                yt_sb[:, 1, 1536 + qc * 128:1536 + (qc + 1) * 128],
                                wp_sb[:, 1, oh * TQ:(oh + 1) * TQ],
                                start=False, stop=True)
                            if oh == 0:
                                nc.scalar.activation(o_sb[:, 0, :], pr[0][:],
                                                     Copy)
                            else:
                                nc.vector.tensor_copy(o_sb[:, 1, :], pr[1][:])
                        trow = 1536 + qc * 128
                        qdma[qc].dma_start(out[trow:trow + 128, 0:TQ],
                                           o_sb[:, 0, :])
                        qdma[(qc + 1) % 4].dma_start(
                            out[trow:trow + 128, TQ:2 * TQ], o_sb[:, 1, :])
    nc.compile()
    return nc


def _emit_k_cc2(nc, mmps, wqkv_sb, xt_sb, k_sb, bqk_sb, p, tt, cc0, st):
    import concourse.mybir as mybir
    f32 = mybir.dt.float32
    if cc0 == 0:
        st["ps"] = mmps.tile([128, TQ], f32, tag="mm", name=f"kd{p}_{tt}")
    for cc in (cc0, cc0 + 1):
        nc.tensor.matmul(st["ps"][:], wqkv_sb[:, 2 + p, cc, :],
                         xt_sb[:, tt, cc, :],
                         start=(cc == 0), stop=(cc == CC - 1))
    if cc0 == CC - 2:
        nc.vector.tensor_scalar_add(
            k_sb[:, p, tt * TQ:(tt + 1) * TQ], st["ps"][:],
            bqk_sb[:, 2 + p:3 + p])


def _emit_proj(nc, mmps, opool, yt_sb, wp_sb, out, qp, ti, oh):
    import concourse.mybir as mybir
    f32 = mybir.dt.float32
    bf16 = mybir.dt.bfloat16
    trow = qp * TQ + ti * 128
    o_ps = mmps.tile([128, TQ], f32, tag="mm", name=f"pj{qp}_{ti}_{oh}")
    for cc in range(2):
        nc.tensor.matmul(o_ps[:], yt_sb[:, cc, trow:trow + 128],
                         wp_sb[:, cc, oh * TQ:(oh + 1) * TQ],
                         start=(cc == 0), stop=(cc == 1))
    o_sb = opool.tile([128, TQ], bf16, tag="osb", name=f"ob{qp}_{ti}_{oh}")
    nc.vector.tensor_copy(o_sb[:], o_ps[:])
    dma_eng = nc.gpsimd if (ti + oh) % 2 == 0 else nc.sync
    dma_eng.dma_start(out[trow:trow + 128, oh * TQ:(oh + 1) * TQ], o_sb[:])


def _get_nc():
    if "nc" not in _CACHE:
        _CACHE["nc"] = _build_nc()
    return _CACHE["nc"]


def _in_maps(x, W_attn, b_attn, W_proj, b_proj):
    import ml_dtypes
    bf = ml_dtypes.bfloat16
    x = np.asarray(x, np.float32).reshape(B, T, C)
    W_attn = np.asarray(W_attn, np.float32)
    b_attn = np.asarray(b_attn, np.float32)
    W_proj = np.asarray(W_proj, np.float32)

    xts = [
        np.ascontiguousarray(
            x[b_].reshape(4, TQ, CC, 128).transpose(3, 0, 2, 1)
        ).astype(bf)
        for b_ in range(B)
    ]
    eye = np.eye(128, dtype=np.float32).astype(bf)

    maps = []
    for i in range(8):
        b_, hg = i // 4, i % 4
        c0 = hg * 256
        cols = np.concatenate([
            np.arange(c0, c0 + 256),
            np.arange(C + c0, C + c0 + 256),
            np.arange(2 * C + c0, 2 * C + c0 + 256),
        ])
        wsel = W_attn[:, cols]
        wqkv_h = np.ascontiguousarray(
            wsel.reshape(CC, 128, 6, 128).transpose(1, 2, 0, 3)
        ).astype(bf)
        bq = b_attn[c0:c0 + 256].reshape(2, 128).T
        bk = b_attn[C + c0:C + c0 + 256].reshape(2, 128).T
        bqk_h = np.ascontiguousarray(
            np.concatenate([bq, bk], axis=1)).astype(np.float32)
        wp_h = np.ascontiguousarray(
            W_proj[c0:c0 + 256, :].reshape(2, 128, C).transpose(1, 0, 2)
        ).astype(bf)
        maps.append({
            "xt": xts[b_], "wqkv": wqkv_h, "bqk": bqk_h,
            "wp": wp_h, "eye": eye,
        })
    return maps


def run(x, W_attn, b_attn, W_proj, b_proj, trace=False):
    from concourse.bass_utils import run_bass_kernel_spmd
    nc = _get_nc()
    maps = _in_maps(x, W_attn, b_attn, W_proj, b_proj)
    res = run_bass_kernel_spmd(nc, maps, list(range(8)), trace=trace)
    out = np.zeros((B, T, C), np.float32)
    for i in range(8):
        out[i // 4] += res.results[i]["out"].astype(np.float32)
    b_attn = np.asarray(b_attn, np.float32)
    b_proj = np.asarray(b_proj, np.float32)
    if b_attn[2 * C:].any() or b_proj.any():
        out += (b_attn[2 * C:] @ np.asarray(W_proj, np.float32)
                + b_proj).astype(np.float32)
    return out, res


def kernel(x, W_attn, b_attn, W_proj, b_proj):
    out, _ = run(x, W_attn, b_attn, W_proj, b_proj, trace=False)
    return out


# revision 26
# speedup vs baseline: 1.0132x; 1.0132x over previous
"""Trainium2 Bass kernel for nn_AudioSelfAttention (B=2, T=2048, C=1024, H=16).

Sharding: batch x head-group tensor parallel. Core i handles batch i//4 and
heads 4*(i%4)..4*(i%4)+3 (2 head pairs) over the full 2048-token batch.
Each core computes q/k/v only for its own 4 heads (no redundant K/V work),
runs attention for its heads over all queries, and produces a partial
output projection out_partial = y_heads @ W_proj[head rows, :] [2048,1024].
The host sums the 4 partials per batch in fp32 (the unshard step).

Attention unit = (head pair, 512-query tile); 8 units cover the 2 pairs x
2048 queries. Per key chunk: S^T via a row-tiled pair of 64-contraction
matmuls (concurrent on disjoint PE row groups; the full 2x2 row+col split
would halve S time but PE quadrant (64,64) is broken on trn2), exp on
ScalarE over [128,1024] PSUM spans, y^T = P^T V with the exp tile
stationary and V+ones moving (65-col matmuls, sumexp in column 64).

ScalarE's exp stream is the bottleneck engine (~1.05us per chunk when the
PE is lightly loaded, ~1.26us when saturated — keep mid-stream PE drips
moderate). K(p0)+Q(p0,0)+V(0..7) run in a dense prologue at full PE rate;
V(8..15), K(p1), the other Q tiles and the output projection are dripped
into the attention units' PE slack. Out-DMAs ride the GpSimd queue: a DMA
wait parked on the scalar queue blocks the exp stream behind it.

Compute dtype: bf16 matmul operands, fp32 PSUM accumulation and softmax
statistics; v/proj biases folded exactly on the host (softmax rows sum to
1); q/k biases applied on-device in the PSUM->SBUF copies.
"""
import numpy as np

_CACHE = {}

B, T, C, H, D = 2, 2048, 1024, 16, 64
CC = C // 128            # 8 contraction chunks
NKT = T // 128           # 16 key chunks
TQ = 512                 # query tile


def _build_nc():
    import concourse.bacc as bacc
    import concourse.tile as tile
    import concourse.mybir as mybir

    f32 = mybir.dt.float32
    bf16 = mybir.dt.bfloat16
    Exp = mybir.ActivationFunctionType.Exp
    Copy = mybir.ActivationFunctionType.Copy

    nc = bacc.Bacc(None, num_devices=8)
    xt = nc.declare_dram_parameter("xt", [128, 4, CC, TQ], bf16, isOutput=False)
    # jc 0,1 = q chunks (pair0, pair1), 2,3 = k, 4,5 = v
    wqkv = nc.declare_dram_parameter("wqkv", [128, 6, CC, 128], bf16, isOutput=False)
    bqk = nc.declare_dram_parameter("bqk", [128, 4], f32, isOutput=False)
    wp = nc.declare_dram_parameter("wp", [128, 2, C], bf16, isOutput=False)
    eye = nc.declare_dram_parameter("eye", [128, 128], bf16, isOutput=False)
    out = nc.declare_dram_parameter("out", [T, C], bf16, isOutput=True)

    with tile.TileContext(nc) as tc:
        with (
            tc.tile_pool(name="big", bufs=1) as big,
            tc.tile_pool(name="pexp", bufs=8) as pexp,
            tc.tile_pool(name="small", bufs=2) as small,
            tc.tile_pool(name="opool", bufs=6) as opool,
            tc.tile_pool(name="mmps", bufs=2, space="PSUM") as mmps,
            tc.tile_pool(name="spool", bufs=2, space="PSUM") as spool,
            tc.tile_pool(name="ypool", bufs=2, space="PSUM") as ypool,
        ):
            # ---- persistent SBUF tensors; DMAs in consumption order.
            # sync: eye, xt0, xt1, wv, xt2, xt3. scalar: wk0, wq0, bqk,
            # wk1, wq1, wp. (wv early — it gates the prologue V tiles.)
            eye_sb = big.tile([128, 128], bf16)
            nc.sync.dma_start(eye_sb[:], eye[:])
            xt_sb = big.tile([128, 4, CC, TQ], bf16)
            nc.sync.dma_start(xt_sb[:, 0, 0:4], xt[:, 0, 0:4])
            nc.gpsimd.dma_start(xt_sb[:, 0, 4:8], xt[:, 0, 4:8])
            wqkv_sb = big.tile([128, 6, CC, 128], bf16)
            nc.scalar.dma_start(wqkv_sb[:, 2:3], wqkv[:, 2:3])
            nc.scalar.dma_start(wqkv_sb[:, 0:1], wqkv[:, 0:1])
            bqk_sb = big.tile([128, 4], f32)
            nc.scalar.dma_start(bqk_sb[:], bqk[:])
            nc.gpsimd.dma_start(wqkv_sb[:, 4:6], wqkv[:, 4:6])
            nc.sync.dma_start(xt_sb[:, 1], xt[:, 1])
            nc.scalar.dma_start(wqkv_sb[:, 3:4], wqkv[:, 3:4])
            nc.scalar.dma_start(wqkv_sb[:, 1:2], wqkv[:, 1:2])
            nc.gpsimd.dma_start(xt_sb[:, 2], xt[:, 2])
            nc.gpsimd.dma_start(xt_sb[:, 3], xt[:, 3])
            wp_sb = big.tile([128, 2, C], bf16)
            nc.scalar.dma_start(wp_sb[:], wp[:])

            q_sb = big.tile([128, 2, T], bf16)
            k_sb = big.tile([128, 2, T], bf16)
            v_sb = big.tile([128, NKT, 4, 72], bf16)
            nc.vector.memset(v_sb[:, :, :, 64:65], 1.0)
            yt_sb = big.tile([128, 2, T], bf16)

            # ---- PE warm-up on uninitialized SBUF during the DMA lead-in
            warm_ps = mmps.tile([128, TQ], f32, tag="mm", name="warm")
            for w in range(20):
                nc.tensor.matmul(warm_ps[:], yt_sb[0:128, 0, 0:128],
                                 yt_sb[:, w % 2, 0:TQ],
                                 start=(w == 0), stop=(w == 19),
                                 skip_group_check=True)

            # ---- K(pair0) ----
            def emit_k_tt(p, tt):
                kps = mmps.tile([128, TQ], f32, tag="mm", name=f"k{p}_{tt}")
                for cc in range(CC):
                    nc.tensor.matmul(kps[:], wqkv_sb[:, 2 + p, cc, :],
                                     xt_sb[:, tt, cc, :],
                                     start=(cc == 0), stop=(cc == CC - 1))
                nc.vector.tensor_scalar_add(
                    k_sb[:, p, tt * TQ:(tt + 1) * TQ], kps[:],
                    bqk_sb[:, 2 + p:3 + p])

            for tt in range(3):
                emit_k_tt(0, tt)

            # ---- Q tiles: 2 accumulation matmuls per call ----
            def emit_q_cc2(p, qt, cc0, st):
                if cc0 == 0:
                    st["ps"] = mmps.tile([128, TQ], f32, tag="mm",
                                         name=f"q{p}_{qt}")
                for cc in (cc0, cc0 + 1):
                    nc.tensor.matmul(st["ps"][:], wqkv_sb[:, p, cc, :],
                                     xt_sb[:, qt, cc, :],
                                     start=(cc == 0), stop=(cc == CC - 1))
                if cc0 == CC - 2:
                    nc.vector.tensor_scalar_add(
                        q_sb[:, p, qt * TQ:(qt + 1) * TQ], st["ps"][:],
                        bqk_sb[:, p:p + 1])

            st0 = {}
            for cc0 in range(0, CC, 2):
                emit_q_cc2(0, 0, cc0, st0)

            # ---- V tiles (joint 256-wide): 0..7 here, 8..15 dripped ----
            def emit_v(tc_i):
                vps = mmps.tile([128, 2, 128], f32, tag="mm", name=f"v{tc_i}")
                for cc in range(CC):
                    nc.tensor.matmul(
                        vps[:],
                        xt_sb[:, tc_i // 4, cc,
                              (tc_i % 4) * 128:(tc_i % 4) * 128 + 128],
                        wqkv_sb[:, 4:6, cc, :],
                        start=(cc == 0), stop=(cc == CC - 1))
                nc.vector.tensor_copy(
                    v_sb[:, tc_i, :, 0:64],
                    vps.rearrange("p j (h f) -> p (j h) f", h=2))

            for tc_i in range(6):
                emit_v(tc_i)

            # ---- attention: 8 units, pair-major (p0 qt0..3, p1 qt0..3) ----
            for u in range(8):
                p, qt = u // 4, u % 4
                qoff = qt * TQ
                last = u == 7

                yA = ypool.tile([128, 4, 66], f32, tag="y", name=f"yA_{u}")
                yB = ypool.tile([128, 4, 66], f32, tag="y", name=f"yB_{u}")
                pe_tiles = {}

                def emit_y(c, yA=yA, yB=yB, p=p):
                    # start=True clears the whole PSUM bank's has_written;
                    # only the first matmul into the bank sets it
                    pe_t = pe_tiles.pop(c)
                    for qc in range(4):
                        nc.tensor.matmul(yA[:, qc, 0:65],
                                         pe_t[:, qc * 128:(qc + 1) * 128],
                                         v_sb[:, c, 2 * p, 0:65],
                                         start=(c == 0 and qc == 0),
                                         stop=(c == NKT - 1 and qc == 3),
                                         skip_group_check=True)
                        nc.tensor.matmul(yB[:, qc, 0:65],
                                         pe_t[:, TQ + qc * 128:TQ + (qc + 1) * 128],
                                         v_sb[:, c, 2 * p + 1, 0:65],
                                         start=(c == 0 and qc == 0),
                                         stop=(c == NKT - 1 and qc == 3),
                                         skip_group_check=True)

                # drip work for this unit, consumed 2 slots per odd kc
                drips = []
                if u == 0:
                    # K(p0,tt3) first (needed by S(12)), then V tiles 6..15;
                    # V(tc) must land before y(tc) (3-chunk lag)
                    stk0 = {}
                    for cc0 in range(0, CC, 2):
                        drips.append(
                            lambda cc0=cc0, stk0=stk0: _emit_k_cc2(
                                nc, mmps, wqkv_sb, xt_sb, k_sb, bqk_sb,
                                0, 3, cc0, stk0))
                    for j in range(10):
                        drips.append(lambda j=j: emit_v(6 + j))
                elif u in (1, 2):
                    # K(pair1): 2 tt per unit, 8 cc matmuls over 4 slots
                    for tt in (2 * (u - 1), 2 * (u - 1) + 1):
                        stk = {}
                        for cc0 in range(0, CC, 2):
                            drips.append(
                                lambda tt=tt, cc0=cc0, stk=stk: _emit_k_cc2(
                                    nc, mmps, wqkv_sb, xt_sb, k_sb, bqk_sb,
                                    1, tt, cc0, stk))
                if u < 7:
                    pn, qn = (u + 1) // 4, (u + 1) % 4
                    stq = {}
                    for cc0 in range(0, CC, 2):
                        drips.append(
                            lambda pn=pn, qn=qn, cc0=cc0, stq=stq:
                            emit_q_cc2(pn, qn, cc0, stq))
                if u >= 5:
                    # proj for qt u-5 (yt cc0+cc1 complete after unit 4+qt)
                    qp = u - 5
                    for ti in range(4):
                        for oh in range(2):
                            drips.append(
                                lambda qp=qp, ti=ti, oh=oh: _emit_proj(
                                    nc, mmps, opool, yt_sb, wp_sb, out,
                                    qp, ti, oh))

                di = iter(drips)

                def drip():
                    fn = next(di, None)
                    if fn is not None:
                        fn()

                for c in range(NKT):
                    koff = c * 128
                    sp = spool.tile([128, 2 * TQ], f32, tag="s",
                                    name=f"s_{u}_{c}")
                    nc.tensor.matmul(sp[:, 0:TQ],
                                     k_sb[0:64, p, koff:koff + 128],
                                     q_sb[0:64, p, qoff:qoff + TQ],
                                     start=True, stop=True)
                    nc.tensor.matmul(sp[:, TQ:2 * TQ],
                                     k_sb[64:128, p, koff:koff + 128],
                                     q_sb[64:128, p, qoff:qoff + TQ],
                                     start=True, stop=True)
                    pe_t = pexp.tile([128, 2 * TQ], bf16, tag="pe",
                                     name=f"pe_{u}_{c}")
                    nc.scalar.activation(pe_t[:], sp[:], Exp, scale=0.125)
                    pe_tiles[c] = pe_t
                    if c % 2 == 1:
                        if c >= 3:
                            emit_y(c - 3)
                            emit_y(c - 2)
                        drip()
                        drip()
                emit_y(NKT - 2)
                emit_y(NKT - 1)
                for _ in range(4):
                    drip()

                # normalization + transpose back to [dim, query]
                rr = small.tile([128, 2, 4], f32, tag="rr", name=f"rr_{u}")
                nc.vector.reciprocal_approx_fast(rr[:, 0, :], yA[:, :, 64])
                nc.vector.reciprocal_approx_fast(rr[:, 1, :], yB[:, :, 64])
                yn = small.tile([128, 4, 128], bf16, tag="yn", name=f"yn_{u}")
                for qc in range(4):
                    if last:
                        nc.scalar.activation(yn[:, qc, 0:64], yA[:, qc, 0:64],
                                             Copy, scale=rr[:, 0, qc:qc + 1])
                    else:
                        nc.vector.tensor_scalar_mul(yn[:, qc, 0:64],
                                                    yA[:, qc, 0:64],
                                                    rr[:, 0, qc:qc + 1])
                    nc.vector.tensor_scalar_mul(yn[:, qc, 64:128],
                                                yB[:, qc, 0:64],
                                                rr[:, 1, qc:qc + 1])
                    if not last:
                        nc.sync.dma_start_transpose(
                            yt_sb[:, p, qoff + qc * 128:qoff + (qc + 1) * 128],
                            yn[:, qc, :])
                if last:
                    # tail: PE transposes + proj qt3 interleaved per qc;
                    # cc0-half proj matmuls pre-run while norm muls drain
                    early = {}
                    for qc in (0, 1):
                        pr = []
                        for oh in range(2):
                            o_ps = (mmps if oh == 0 else spool).tile(
                                [128, TQ], f32, tag="mm" if oh == 0 else "s",
                                name=f"op_{qc}_{oh}")
                            nc.tensor.matmul(
                                o_ps[:],
                                yt_sb[:, 0, 1536 + qc * 128:1536 + (qc + 1) * 128],
                                wp_sb[:, 0, oh * TQ:(oh + 1) * TQ],
                                start=True, stop=False)
                            pr.append(o_ps)
                        early[qc] = pr
                    qdma = [nc.sync, nc.gpsimd, nc.scalar, nc.gpsimd]
                    for qc in range(4):
                        tp = ypool.tile([128, 128], bf16, tag="y",
                                        name=f"tp_{qc}")
                        nc.tensor.transpose(tp[:], yn[:, qc, :], eye_sb[:])
                        nc.scalar.activation(
                            yt_sb[:, 1, 1536 + qc * 128:1536 + (qc + 1) * 128],
                            tp[:], Copy)
                        if qc in early:
                            pr = early.pop(qc)
                        else:
                            pr = []
                            for oh in range(2):
                                o_ps = (mmps if oh == 0 else spool).tile(
                                    [128, TQ], f32,
                                    tag="mm" if oh == 0 else "s",
                                    name=f"op_{qc}_{oh}")
                                nc.tensor.matmul(
                                    o_ps[:],
                                    yt_sb[:, 0, 1536 + qc * 128:1536 + (qc + 1) * 128],
                                    wp_sb[:, 0, oh * TQ:(oh + 1) * TQ],
                                    start=True, stop=False)
                                pr.append(o_ps)
                        o_sb = small.tile([128, 2, TQ], bf16, tag="osb",
                                          name=f"osb_{qc}")
                        for oh in range(2):
                            nc.tensor.matmul(
                                pr[oh][:],
                                yt_sb[:, 1, 1536 + qc * 128:1536 + (qc + 1) * 128],
                                wp_sb[:, 1, oh * TQ:(oh + 1) * TQ],
                                start=False, stop=True)
                            if oh == 0:
                                nc.scalar.activation(o_sb[:, 0, :], pr[0][:],
                                                     Copy)
                            else:
                                nc.vector.tensor_copy(o_sb[:, 1, :], pr[1][:])
                        trow = 1536 + qc * 128
                        qdma[qc].dma_start(out[trow:trow + 128, 0:TQ],
                                           o_sb[:, 0, :])
                        qdma[(qc + 1) % 4].dma_start(
                            out[trow:trow + 128, TQ:2 * TQ], o_sb[:, 1, :])
    nc.compile()
    return nc


def _emit_k_cc2(nc, mmps, wqkv_sb, xt_sb, k_sb, bqk_sb, p, tt, cc0, st):
    import concourse.mybir as mybir
    f32 = mybir.dt.float32
    if cc0 == 0:
        st["ps"] = mmps.tile([128, TQ], f32, tag="mm", name=f"kd{p}_{tt}")
    for cc in (cc0, cc0 + 1):
        nc.tensor.matmul(st["ps"][:], wqkv_sb[:, 2 + p, cc, :],
                         xt_sb[:, tt, cc, :],
                         start=(cc == 0), stop=(cc == CC - 1))
    if cc0 == CC - 2:
        nc.vector.tensor_scalar_add(
            k_sb[:, p, tt * TQ:(tt + 1) * TQ], st["ps"][:],
            bqk_sb[:, 2 + p:3 + p])


def _emit_proj(nc, mmps, opool, yt_sb, wp_sb, out, qp, ti, oh):
    import concourse.mybir as mybir
    f32 = mybir.dt.float32
    bf16 = mybir.dt.bfloat16
    trow = qp * TQ + ti * 128
    o_ps = mmps.tile([128, TQ], f32, tag="mm", name=f"pj{qp}_{ti}_{oh}")
    for cc in range(2):
        nc.tensor.matmul(o_ps[:], yt_sb[:, cc, trow:trow + 128],
                         wp_sb[:, cc, oh * TQ:(oh + 1) * TQ],
                         start=(cc == 0), stop=(cc == 1))
    o_sb = opool.tile([128, TQ], bf16, tag="osb", name=f"ob{qp}_{ti}_{oh}")
    nc.vector.tensor_copy(o_sb[:], o_ps[:])
    dma_eng = nc.gpsimd if (ti + oh) % 2 == 0 else nc.sync
    dma_eng.dma_start(out[trow:trow + 128, oh * TQ:(oh + 1) * TQ], o_sb[:])


def _get_nc():
    if "nc" not in _CACHE:
        _CACHE["nc"] = _build_nc()
    return _CACHE["nc"]


def _in_maps(x, W_attn, b_attn, W_proj, b_proj):
    import ml_dtypes
    bf = ml_dtypes.bfloat16
    x = np.asarray(x, np.float32).reshape(B, T, C)
    W_attn = np.asarray(W_attn, np.float32)
    b_attn = np.asarray(b_attn, np.float32)
    W_proj = np.asarray(W_proj, np.float32)

    xts = [
        np.ascontiguousarray(
            x[b_].reshape(4, TQ, CC, 128).transpose(3, 0, 2, 1)
        ).astype(bf)
        for b_ in range(B)
    ]
    eye = np.eye(128, dtype=np.float32).astype(bf)

    maps = []
    for i in range(8):
        b_, hg = i // 4, i % 4
        c0 = hg * 256
        cols = np.concatenate([
            np.arange(c0, c0 + 256),
            np.arange(C + c0, C + c0 + 256),
            np.arange(2 * C + c0, 2 * C + c0 + 256),
        ])
        wsel = W_attn[:, cols]
        wqkv_h = np.ascontiguousarray(
            wsel.reshape(CC, 128, 6, 128).transpose(1, 2, 0, 3)
        ).astype(bf)
        bq = b_attn[c0:c0 + 256].reshape(2, 128).T
        bk = b_attn[C + c0:C + c0 + 256].reshape(2, 128).T
        bqk_h = np.ascontiguousarray(
            np.concatenate([bq, bk], axis=1)).astype(np.float32)
        wp_h = np.ascontiguousarray(
            W_proj[c0:c0 + 256, :].reshape(2, 128, C).transpose(1, 0, 2)
        ).astype(bf)
        maps.append({
            "xt": xts[b_], "wqkv": wqkv_h, "bqk": bqk_h,
            "wp": wp_h, "eye": eye,
        })
    return maps


def run(x, W_attn, b_attn, W_proj, b_proj, trace=False):
    from concourse.bass_utils import run_bass_kernel_spmd
    nc = _get_nc()
    maps = _in_maps(x, W_attn, b_attn, W_proj, b_proj)
    res = run_bass_kernel_spmd(nc, maps, list(range(8)), trace=trace)
    out = np.zeros((B, T, C), np.float32)
    for i in range(8):
        out[i // 4] += res.results[i]["out"].astype(np.float32)
    b_attn = np.asarray(b_attn, np.float32)
    b_proj = np.asarray(b_proj, np.float32)
    if b_attn[2 * C:].any() or b_proj.any():
        out += (b_attn[2 * C:] @ np.asarray(W_proj, np.float32)
                + b_proj).astype(np.float32)
    return out, res


def kernel(x, W_attn, b_attn, W_proj, b_proj):
    out, _ = run(x, W_attn, b_attn, W_proj, b_proj, trace=False)
    return out


# revision 27
# speedup vs baseline: 1.0275x; 1.0141x over previous
"""Trainium2 Bass kernel for nn_AudioSelfAttention (B=2, T=2048, C=1024, H=16).

Sharding: batch x head-group tensor parallel. Core i handles batch i//4 and
heads 4*(i%4)..4*(i%4)+3 (2 head pairs) over the full 2048-token batch.
Each core computes q/k/v only for its own 4 heads (no redundant K/V work),
runs attention for its heads over all queries, and produces a partial
output projection out_partial = y_heads @ W_proj[head rows, :] [2048,1024].
The host sums the 4 partials per batch in fp32 (the unshard step).

Attention unit = (head pair, 512-query tile); 8 units cover the 2 pairs x
2048 queries. Per key chunk: S^T via a row-tiled pair of 64-contraction
matmuls (concurrent on disjoint PE row groups; the full 2x2 row+col split
would halve S time but PE quadrant (64,64) is broken on trn2), exp on
ScalarE over [128,1024] PSUM spans, y^T = P^T V with the exp tile
stationary and V+ones moving (65-col matmuls, sumexp in column 64).

ScalarE's exp stream is the bottleneck engine (~1.05us per chunk when the
PE is lightly loaded, ~1.26us when saturated — keep mid-stream PE drips
moderate). K(p0)+Q(p0,0)+V(0..7) run in a dense prologue at full PE rate;
V(8..15), K(p1), the other Q tiles and the output projection are dripped
into the attention units' PE slack. Out-DMAs ride the GpSimd queue: a DMA
wait parked on the scalar queue blocks the exp stream behind it.

Compute dtype: bf16 matmul operands, fp32 PSUM accumulation and softmax
statistics; v/proj biases folded exactly on the host (softmax rows sum to
1); q/k biases applied on-device in the PSUM->SBUF copies.
"""
import numpy as np

_CACHE = {}

B, T, C, H, D = 2, 2048, 1024, 16, 64
CC = C // 128            # 8 contraction chunks
NKT = T // 128           # 16 key chunks
TQ = 512                 # query tile


def _build_nc():
    import concourse.bacc as bacc
    import concourse.tile as tile
    import concourse.mybir as mybir

    f32 = mybir.dt.float32
    bf16 = mybir.dt.bfloat16
    Exp = mybir.ActivationFunctionType.Exp
    Copy = mybir.ActivationFunctionType.Copy

    nc = bacc.Bacc(None, num_devices=8)
    xt = nc.declare_dram_parameter("xt", [128, 4, CC, TQ], bf16, isOutput=False)
    # jc 0,1 = q chunks (pair0, pair1), 2,3 = k, 4,5 = v
    wqkv = nc.declare_dram_parameter("wqkv", [128, 6, CC, 128], bf16, isOutput=False)
    bqk = nc.declare_dram_parameter("bqk", [128, 4], f32, isOutput=False)
    wp = nc.declare_dram_parameter("wp", [128, 2, C], bf16, isOutput=False)
    eye = nc.declare_dram_parameter("eye", [128, 128], bf16, isOutput=False)
    out = nc.declare_dram_parameter("out", [T, C], bf16, isOutput=True)

    with tile.TileContext(nc) as tc:
        with (
            tc.tile_pool(name="big", bufs=1) as big,
            tc.tile_pool(name="pexp", bufs=8) as pexp,
            tc.tile_pool(name="small", bufs=2) as small,
            tc.tile_pool(name="opool", bufs=6) as opool,
            tc.tile_pool(name="mmps", bufs=2, space="PSUM") as mmps,
            tc.tile_pool(name="spool", bufs=2, space="PSUM") as spool,
            tc.tile_pool(name="ypool", bufs=2, space="PSUM") as ypool,
        ):
            # ---- persistent SBUF tensors; DMAs in consumption order.
            # sync: eye, xt0, xt1, wv, xt2, xt3. scalar: wk0, wq0, bqk,
            # wk1, wq1, wp. (wv early — it gates the prologue V tiles.)
            eye_sb = big.tile([128, 128], bf16)
            nc.sync.dma_start(eye_sb[:], eye[:])
            xt_sb = big.tile([128, 4, CC, TQ], bf16)
            wqkv_sb = big.tile([128, 6, CC, 128], bf16)
            bqk_sb = big.tile([128, 4], f32)
            wp_sb = big.tile([128, 2, C], bf16)
            # token tiles split across sync+gpsimd halves (arrive in tt
            # order); weights on scalar in consumption order
            for tt in range(4):
                nc.sync.dma_start(xt_sb[:, tt, 0:4], xt[:, tt, 0:4])
                nc.gpsimd.dma_start(xt_sb[:, tt, 4:8], xt[:, tt, 4:8])
            nc.scalar.dma_start(wqkv_sb[:, 2:3], wqkv[:, 2:3])
            nc.scalar.dma_start(wqkv_sb[:, 0:1], wqkv[:, 0:1])
            nc.scalar.dma_start(bqk_sb[:], bqk[:])
            nc.scalar.dma_start(wqkv_sb[:, 4:6], wqkv[:, 4:6])
            nc.scalar.dma_start(wqkv_sb[:, 3:4], wqkv[:, 3:4])
            nc.scalar.dma_start(wqkv_sb[:, 1:2], wqkv[:, 1:2])
            nc.scalar.dma_start(wp_sb[:], wp[:])

            q_sb = big.tile([128, 2, T], bf16)
            k_sb = big.tile([128, 2, T], bf16)
            v_sb = big.tile([128, NKT, 4, 72], bf16)
            nc.vector.memset(v_sb[:, :, :, 64:65], 1.0)
            yt_sb = big.tile([128, 2, T], bf16)

            # ---- PE warm-up on uninitialized SBUF during the DMA lead-in
            warm_ps = mmps.tile([128, TQ], f32, tag="mm", name="warm")
            for w in range(20):
                nc.tensor.matmul(warm_ps[:], yt_sb[0:128, 0, 0:128],
                                 yt_sb[:, w % 2, 0:TQ],
                                 start=(w == 0), stop=(w == 19),
                                 skip_group_check=True)

            # ---- K(pair0) ----
            def emit_k_tt(p, tt):
                kps = mmps.tile([128, TQ], f32, tag="mm", name=f"k{p}_{tt}")
                for cc in range(CC):
                    nc.tensor.matmul(kps[:], wqkv_sb[:, 2 + p, cc, :],
                                     xt_sb[:, tt, cc, :],
                                     start=(cc == 0), stop=(cc == CC - 1))
                nc.vector.tensor_scalar_add(
                    k_sb[:, p, tt * TQ:(tt + 1) * TQ], kps[:],
                    bqk_sb[:, 2 + p:3 + p])

            for tt in range(2):
                emit_k_tt(0, tt)

            # ---- Q tiles: 2 accumulation matmuls per call ----
            def emit_q_cc2(p, qt, cc0, st):
                if cc0 == 0:
                    st["ps"] = mmps.tile([128, TQ], f32, tag="mm",
                                         name=f"q{p}_{qt}")
                for cc in (cc0, cc0 + 1):
                    nc.tensor.matmul(st["ps"][:], wqkv_sb[:, p, cc, :],
                                     xt_sb[:, qt, cc, :],
                                     start=(cc == 0), stop=(cc == CC - 1))
                if cc0 == CC - 2:
                    nc.vector.tensor_scalar_add(
                        q_sb[:, p, qt * TQ:(qt + 1) * TQ], st["ps"][:],
                        bqk_sb[:, p:p + 1])

            st0 = {}
            for cc0 in range(0, CC, 2):
                emit_q_cc2(0, 0, cc0, st0)

            # ---- V tiles (joint 256-wide): 0..7 here, 8..15 dripped ----
            def emit_v(tc_i):
                vps = mmps.tile([128, 2, 128], f32, tag="mm", name=f"v{tc_i}")
                for cc in range(CC):
                    nc.tensor.matmul(
                        vps[:],
                        xt_sb[:, tc_i // 4, cc,
                              (tc_i % 4) * 128:(tc_i % 4) * 128 + 128],
                        wqkv_sb[:, 4:6, cc, :],
                        start=(cc == 0), stop=(cc == CC - 1))
                nc.vector.tensor_copy(
                    v_sb[:, tc_i, :, 0:64],
                    vps.rearrange("p j (h f) -> p (j h) f", h=2))

            for tc_i in range(6):
                emit_v(tc_i)

            # ---- attention: 8 units, pair-major (p0 qt0..3, p1 qt0..3) ----
            for u in range(8):
                p, qt = u // 4, u % 4
                qoff = qt * TQ
                last = u == 7

                yA = ypool.tile([128, 4, 66], f32, tag="y", name=f"yA_{u}")
                yB = ypool.tile([128, 4, 66], f32, tag="y", name=f"yB_{u}")
                pe_tiles = {}

                def emit_y(c, yA=yA, yB=yB, p=p):
                    # start=True clears the whole PSUM bank's has_written;
                    # only the first matmul into the bank sets it
                    pe_t = pe_tiles.pop(c)
                    for qc in range(4):
                        nc.tensor.matmul(yA[:, qc, 0:65],
                                         pe_t[:, qc * 128:(qc + 1) * 128],
                                         v_sb[:, c, 2 * p, 0:65],
                                         start=(c == 0 and qc == 0),
                                         stop=(c == NKT - 1 and qc == 3),
                                         skip_group_check=True)
                        nc.tensor.matmul(yB[:, qc, 0:65],
                                         pe_t[:, TQ + qc * 128:TQ + (qc + 1) * 128],
                                         v_sb[:, c, 2 * p + 1, 0:65],
                                         start=(c == 0 and qc == 0),
                                         stop=(c == NKT - 1 and qc == 3),
                                         skip_group_check=True)

                # drip work for this unit, consumed 2 slots per odd kc
                drips = []
                if u == 0:
                    # K(p0) tt2 then tt3 (by S(8)/S(12)), then V tiles
                    # 6..15 (before each y at the unit-0 lag of 5)
                    for tt3 in (2, 3):
                        stk0 = {}
                        for cc0 in range(0, CC, 2):
                            drips.append(
                                lambda tt3=tt3, cc0=cc0, stk0=stk0:
                                _emit_k_cc2(nc, mmps, wqkv_sb, xt_sb, k_sb,
                                            bqk_sb, 0, tt3, cc0, stk0))
                    for j in range(10):
                        drips.append(lambda j=j: emit_v(6 + j))
                elif u in (1, 2):
                    # K(pair1): 2 tt per unit, 8 cc matmuls over 4 slots
                    for tt in (2 * (u - 1), 2 * (u - 1) + 1):
                        stk = {}
                        for cc0 in range(0, CC, 2):
                            drips.append(
                                lambda tt=tt, cc0=cc0, stk=stk: _emit_k_cc2(
                                    nc, mmps, wqkv_sb, xt_sb, k_sb, bqk_sb,
                                    1, tt, cc0, stk))
                if u < 7:
                    pn, qn = (u + 1) // 4, (u + 1) % 4
                    stq = {}
                    for cc0 in range(0, CC, 2):
                        drips.append(
                            lambda pn=pn, qn=qn, cc0=cc0, stq=stq:
                            emit_q_cc2(pn, qn, cc0, stq))
                if u >= 5:
                    # proj for qt u-5 (yt cc0+cc1 complete after unit 4+qt)
                    qp = u - 5
                    for ti in range(4):
                        for oh in range(2):
                            drips.append(
                                lambda qp=qp, ti=ti, oh=oh: _emit_proj(
                                    nc, mmps, opool, yt_sb, wp_sb, out,
                                    qp, ti, oh))

                di = iter(drips)

                def drip():
                    fn = next(di, None)
                    if fn is not None:
                        fn()

                lag = 5 if u == 0 else 3
                for c in range(NKT):
                    koff = c * 128
                    sp = spool.tile([128, 2 * TQ], f32, tag="s",
                                    name=f"s_{u}_{c}")
                    nc.tensor.matmul(sp[:, 0:TQ],
                                     k_sb[0:64, p, koff:koff + 128],
                                     q_sb[0:64, p, qoff:qoff + TQ],
                                     start=True, stop=True)
                    nc.tensor.matmul(sp[:, TQ:2 * TQ],
                                     k_sb[64:128, p, koff:koff + 128],
                                     q_sb[64:128, p, qoff:qoff + TQ],
                                     start=True, stop=True)
                    pe_t = pexp.tile([128, 2 * TQ], bf16, tag="pe",
                                     name=f"pe_{u}_{c}")
                    nc.scalar.activation(pe_t[:], sp[:], Exp, scale=0.125)
                    pe_tiles[c] = pe_t
                    if c % 2 == 1:
                        if c >= lag:
                            emit_y(c - lag)
                            emit_y(c - lag + 1)
                        drip()
                        drip()
                for j in range(NKT - lag + 1, NKT):
                    emit_y(j)
                    drip()
                for _ in range(2):
                    drip()

                # normalization + transpose back to [dim, query]
                rr = small.tile([128, 2, 4], f32, tag="rr", name=f"rr_{u}")
                nc.vector.reciprocal_approx_fast(rr[:, 0, :], yA[:, :, 64])
                nc.vector.reciprocal_approx_fast(rr[:, 1, :], yB[:, :, 64])
                yn = small.tile([128, 4, 128], bf16, tag="yn", name=f"yn_{u}")
                for qc in range(4):
                    if last:
                        nc.scalar.activation(yn[:, qc, 0:64], yA[:, qc, 0:64],
                                             Copy, scale=rr[:, 0, qc:qc + 1])
                    else:
                        nc.vector.tensor_scalar_mul(yn[:, qc, 0:64],
                                                    yA[:, qc, 0:64],
                                                    rr[:, 0, qc:qc + 1])
                    nc.vector.tensor_scalar_mul(yn[:, qc, 64:128],
                                                yB[:, qc, 0:64],
                                                rr[:, 1, qc:qc + 1])
                    if not last:
                        nc.sync.dma_start_transpose(
                            yt_sb[:, p, qoff + qc * 128:qoff + (qc + 1) * 128],
                            yn[:, qc, :])
                if last:
                    # tail: PE transposes + proj qt3 interleaved per qc;
                    # cc0-half proj matmuls pre-run while norm muls drain
                    early = {}
                    for qc in (0, 1):
                        pr = []
                        for oh in range(2):
                            o_ps = (mmps if oh == 0 else spool).tile(
                                [128, TQ], f32, tag="mm" if oh == 0 else "s",
                                name=f"op_{qc}_{oh}")
                            nc.tensor.matmul(
                                o_ps[:],
                                yt_sb[:, 0, 1536 + qc * 128:1536 + (qc + 1) * 128],
                                wp_sb[:, 0, oh * TQ:(oh + 1) * TQ],
                                start=True, stop=False)
                            pr.append(o_ps)
                        early[qc] = pr
                    qdma = [nc.sync, nc.gpsimd, nc.scalar, nc.gpsimd]
                    for qc in range(4):
                        tp = ypool.tile([128, 128], bf16, tag="y",
                                        name=f"tp_{qc}")
                        nc.tensor.transpose(tp[:], yn[:, qc, :], eye_sb[:])
                        nc.scalar.activation(
                            yt_sb[:, 1, 1536 + qc * 128:1536 + (qc + 1) * 128],
                            tp[:], Copy)
                        if qc in early:
                            pr = early.pop(qc)
                        else:
                            pr = []
                            for oh in range(2):
                                o_ps = (mmps if oh == 0 else spool).tile(
                                    [128, TQ], f32,
                                    tag="mm" if oh == 0 else "s",
                                    name=f"op_{qc}_{oh}")
                                nc.tensor.matmul(
                                    o_ps[:],
                                    yt_sb[:, 0, 1536 + qc * 128:1536 + (qc + 1) * 128],
                                    wp_sb[:, 0, oh * TQ:(oh + 1) * TQ],
                                    start=True, stop=False)
                                pr.append(o_ps)
                        o_sb = small.tile([128, 2, TQ], bf16, tag="osb",
                                          name=f"osb_{qc}")
                        for oh in range(2):
                            nc.tensor.matmul(
                                pr[oh][:],
                                yt_sb[:, 1, 1536 + qc * 128:1536 + (qc + 1) * 128],
                                wp_sb[:, 1, oh * TQ:(oh + 1) * TQ],
                                start=False, stop=True)
                            if oh == 0:
                                nc.scalar.activation(o_sb[:, 0, :], pr[0][:],
                                                     Copy)
                            else:
                                nc.vector.tensor_copy(o_sb[:, 1, :], pr[1][:])
                        trow = 1536 + qc * 128
                        qdma[qc].dma_start(out[trow:trow + 128, 0:TQ],
                                           o_sb[:, 0, :])
                        qdma[(qc + 1) % 4].dma_start(
                            out[trow:trow + 128, TQ:2 * TQ], o_sb[:, 1, :])
    nc.compile()
    return nc


def _emit_k_cc2(nc, mmps, wqkv_sb, xt_sb, k_sb, bqk_sb, p, tt, cc0, st):
    import concourse.mybir as mybir
    f32 = mybir.dt.float32
    if cc0 == 0:
        st["ps"] = mmps.tile([128, TQ], f32, tag="mm", name=f"kd{p}_{tt}")
    for cc in (cc0, cc0 + 1):
        nc.tensor.matmul(st["ps"][:], wqkv_sb[:, 2 + p, cc, :],
                         xt_sb[:, tt, cc, :],
                         start=(cc == 0), stop=(cc == CC - 1))
    if cc0 == CC - 2:
        nc.vector.tensor_scalar_add(
            k_sb[:, p, tt * TQ:(tt + 1) * TQ], st["ps"][:],
            bqk_sb[:, 2 + p:3 + p])


def _emit_proj(nc, mmps, opool, yt_sb, wp_sb, out, qp, ti, oh):
    import concourse.mybir as mybir
    f32 = mybir.dt.float32
    bf16 = mybir.dt.bfloat16
    trow = qp * TQ + ti * 128
    o_ps = mmps.tile([128, TQ], f32, tag="mm", name=f"pj{qp}_{ti}_{oh}")
    for cc in range(2):
        nc.tensor.matmul(o_ps[:], yt_sb[:, cc, trow:trow + 128],
                         wp_sb[:, cc, oh * TQ:(oh + 1) * TQ],
                         start=(cc == 0), stop=(cc == 1))
    o_sb = opool.tile([128, TQ], bf16, tag="osb", name=f"ob{qp}_{ti}_{oh}")
    nc.vector.tensor_copy(o_sb[:], o_ps[:])
    dma_eng = nc.gpsimd if (ti + oh) % 2 == 0 else nc.sync
    dma_eng.dma_start(out[trow:trow + 128, oh * TQ:(oh + 1) * TQ], o_sb[:])


def _get_nc():
    if "nc" not in _CACHE:
        _CACHE["nc"] = _build_nc()
    return _CACHE["nc"]


def _in_maps(x, W_attn, b_attn, W_proj, b_proj):
    import ml_dtypes
    bf = ml_dtypes.bfloat16
    x = np.asarray(x, np.float32).reshape(B, T, C)
    W_attn = np.asarray(W_attn, np.float32)
    b_attn = np.asarray(b_attn, np.float32)
    W_proj = np.asarray(W_proj, np.float32)

    xts = [
        np.ascontiguousarray(
            x[b_].reshape(4, TQ, CC, 128).transpose(3, 0, 2, 1)
        ).astype(bf)
        for b_ in range(B)
    ]
    eye = np.eye(128, dtype=np.float32).astype(bf)

    maps = []
    for i in range(8):
        b_, hg = i // 4, i % 4
        c0 = hg * 256
        cols = np.concatenate([
            np.arange(c0, c0 + 256),
            np.arange(C + c0, C + c0 + 256),
            np.arange(2 * C + c0, 2 * C + c0 + 256),
        ])
        wsel = W_attn[:, cols]
        wqkv_h = np.ascontiguousarray(
            wsel.reshape(CC, 128, 6, 128).transpose(1, 2, 0, 3)
        ).astype(bf)
        bq = b_attn[c0:c0 + 256].reshape(2, 128).T
        bk = b_attn[C + c0:C + c0 + 256].reshape(2, 128).T
        bqk_h = np.ascontiguousarray(
            np.concatenate([bq, bk], axis=1)).astype(np.float32)
        wp_h = np.ascontiguousarray(
            W_proj[c0:c0 + 256, :].reshape(2, 128, C).transpose(1, 0, 2)
        ).astype(bf)
        maps.append({
            "xt": xts[b_], "wqkv": wqkv_h, "bqk": bqk_h,
            "wp": wp_h, "eye": eye,
        })
    return maps


def run(x, W_attn, b_attn, W_proj, b_proj, trace=False):
    from concourse.bass_utils import run_bass_kernel_spmd
    nc = _get_nc()
    maps = _in_maps(x, W_attn, b_attn, W_proj, b_proj)
    res = run_bass_kernel_spmd(nc, maps, list(range(8)), trace=trace)
    out = np.zeros((B, T, C), np.float32)
    for i in range(8):
        out[i // 4] += res.results[i]["out"].astype(np.float32)
    b_attn = np.asarray(b_attn, np.float32)
    b_proj = np.asarray(b_proj, np.float32)
    if b_attn[2 * C:].any() or b_proj.any():
        out += (b_attn[2 * C:] @ np.asarray(W_proj, np.float32)
                + b_proj).astype(np.float32)
    return out, res


def kernel(x, W_attn, b_attn, W_proj, b_proj):
    out, _ = run(x, W_attn, b_attn, W_proj, b_proj, trace=False)
    return out
